# revision 1
# baseline (speedup 1.0000x reference)
"""Trainium2 Bass kernel for a single DeBERTa-style attention head.

Problem shapes (hardcoded):
  B=8, S=2048, E=768(n_embed), H=64(head)
  q = I @ Wq + bq ; k = x @ Wk + bk ; v = x @ Wv + bv
  w = (q @ k^T) / sqrt(E) ; w = where(mask==0, -1e9, w)
  scores = softmax(w, axis=-1) ; out = scores @ v

Sharding: data-parallel over batch B across the 8 NeuronCores (one batch
element per core, identical SPMD program). Host-side (inside kernel()) the
per-core slices are laid out transposed (I^T, x^T, mask^T) so the device
never has to transpose bulk data: PE transposes cost ~300ns per 128x128
block and suppress the PE clock-gate warmup, which dominated the v1 profile.

Per-core dataflow (bf16 operands, fp32 PSUM accumulation):
  1. Cast-DMA (SWDGE fp32->bf16) I^T, x^T into SBUF with embed on partitions.
  2. qT,kT [64,2048] = Wq/Wk-chunk (stationary) x I^T/x^T (streaming) + rank-1
     bias matmul; v per k-chunk + bias + a ones column (v_aug) so the softmax
     denominator falls out of the second matmul's extra output column.
  3. k-chunk-major attention: w^T-chunk [128k, q] = kT-chunk^T @ qT;
     e = exp(w^T * 1/sqrt(E)) on ACT straight from PSUM (no row max needed:
     |w/sqrt(E)| is O(1) so exp cannot overflow, and softmax is
     shift-invariant); s^T = e * mask^T (mask int32 DMA-cast to bf16;
     multiplicative masking matches the reference's -1e9 additive mask, which
     underflows to exactly 0 after softmax); ctx[q-chunk, 0:65] accumulates
     s^T-chunk^T @ v_aug over all 16 k-chunks in PSUM.
  4. out = ctx[:,0:64] * (1/ctx[:,64]).
"""

import math
from contextlib import ExitStack

import numpy as np

import concourse.bass as bass
import concourse.tile as tile
import concourse.mybir as mybir
from concourse import bacc
from concourse.bass_utils import run_bass_kernel_spmd

B, S, E, H = 8, 2048, 768, 64
N_CORES = 8
SC = S // 128   # 16 seq chunks
EC = E // 128   # 6 embed chunks
SCALE = 1.0 / math.sqrt(E)

F32 = mybir.dt.float32
BF16 = mybir.dt.bfloat16
I32 = mybir.dt.int32
AF = mybir.ActivationFunctionType
ALU = mybir.AluOpType

_cache = {}


def _build_program():
    nc = bacc.Bacc("TRN2", target_bir_lowering=False, debug=False)

    # Host feeds these already transposed: IT/XT are [E, S], maskT is [S, S]
    # with [k, q] indexing, packed to uint8 (values are 0/1, so the cast is
    # lossless) — 4MB of HBM reads instead of 16MB, expanded to bf16 by the
    # cast-DMA on the way into SBUF.
    dIT = nc.dram_tensor("IT", [E, S], F32, kind="ExternalInput")
    dXT = nc.dram_tensor("XT", [E, S], F32, kind="ExternalInput")
    dmT = nc.dram_tensor("maskT", [S, S], mybir.dt.uint8, kind="ExternalInput")
    # weights host-packed into one contiguous [E, 3H] bf16 tensor and biases
    # into one [H, 2H] f32 tensor (bq/bk broadcast along rows so each
    # partition moves one contiguous run; only col 0 / col H is read)
    dW = nc.dram_tensor("Wpack", [E, 3 * H], BF16, kind="ExternalInput")
    dB = nc.dram_tensor("bpack", [H, 2 * H], F32, kind="ExternalInput")
    dbv = nc.dram_tensor("bv", [1, H], BF16, kind="ExternalInput")
    dout = nc.dram_tensor("out", [S, H], F32, kind="ExternalOutput")

    with tile.TileContext(nc) as tc, ExitStack() as ctx:
        singles = ctx.enter_context(tc.tile_pool(name="singles", bufs=1))

        # SWDGE FIFO order = consumption order: all of I (q path), then x in
        # halves (k/v path), then the mask chunks. The kernel end is paced by
        # the last mask byte, with compute draining right behind it.
        IT = singles.tile([128, EC, S], BF16, tag="IT")
        XT = singles.tile([128, EC, S], BF16, tag="XT")
        nc.gpsimd.dma_start(
            out=IT, in_=dIT.ap().rearrange("(ec p) s -> p ec s", p=128)
        )
        for lo, hi in ((0, S // 2), (S // 2, S)):
            nc.gpsimd.dma_start(
                out=XT[:, :, lo:hi],
                in_=dXT.ap()[:, lo:hi].rearrange("(ec p) s -> p ec s", p=128),
            )

        ones_row = singles.tile([1, S], BF16, tag="ones")
        nc.vector.memset(ones_row, 1.0)

        w_all = singles.tile([128, EC, 3 * H], BF16, tag="Wpack")
        nc.sync.dma_start(
            out=w_all, in_=dW.ap().rearrange("(ec p) h -> p ec h", p=128)
        )
        w_sb = {
            "Wq": w_all[:, :, 0:H],
            "Wk": w_all[:, :, H:2 * H],
            "Wv": w_all[:, :, 2 * H:3 * H],
        }
        b_all = singles.tile([H, 2 * H], F32, tag="bpack")
        nc.sync.dma_start(out=b_all, in_=dB.ap())
        bv_t = singles.tile([1, H], BF16, tag="bv")
        nc.sync.dma_start(out=bv_t, in_=dbv.ap())
        b_sb = {"bq": b_all[:, 0:1], "bk": b_all[:, H:H + 1], "bv": bv_t}

        qT = singles.tile([64, S], BF16, tag="qT")
        kT = singles.tile([64, S], BF16, tag="kT")
        vA = singles.tile([128, SC, 66], BF16, tag="vA")

        # whole mask^T resident (bf16, 64KB/partition), filled by upfront
        # cast-DMAs from the uint8 source — no consumer-slot gating on the
        # stream, which otherwise serializes DMA latency into the TT chain
        maskT_all = singles.tile([128, SC, S], BF16, tag="maskT")
        for ki in range(0, SC, 2):
            nc.gpsimd.dma_start(
                out=maskT_all[:, ki:ki + 2, :],
                in_=dmT.ap()[ki * 128:(ki + 2) * 128, :].rearrange(
                    "(t p) q -> p t q", p=128
                ),
            )

        psw = ctx.enter_context(tc.tile_pool(name="psw", bufs=2, space="PSUM"))
        sp = ctx.enter_context(tc.tile_pool(name="sp", bufs=9))
        eep = ctx.enter_context(tc.tile_pool(name="eep", bufs=3))
        outp = ctx.enter_context(tc.tile_pool(name="outp", bufs=1))

        def emit_score(ki):
            """w^T-chunk -> exp -> mask multiply; returns the sT tile.

            Both w halves are emitted before both exps (and both exps before
            both multiplies) so each engine sees its two ops back-to-back —
            the inter-op pipeline drain overlaps the other half's work on the
            neighbouring engines instead of serializing the chain."""
            maskT_sb = maskT_all[:, ki, :]
            sT_sb = sp.tile([128, S], BF16, tag="sT")
            wps = []
            for hh in range(2):
                wp = psw.tile([128, 1024], F32, tag="w")
                for nb in range(2):
                    nc.tensor.matmul(
                        wp[:, nb * 512:(nb + 1) * 512],
                        lhsT=kT[:, ki * 128:(ki + 1) * 128],
                        rhs=qT[:, (hh * 2 + nb) * 512:(hh * 2 + nb + 1) * 512],
                        start=True,
                        stop=True,
                    )
                wps.append(wp)
            e_sbs = []
            for hh in range(2):
                e_sb = eep.tile([128, 1024], BF16, tag="e")
                nc.scalar.activation(e_sb, wps[hh], AF.Exp, scale=SCALE)
                e_sbs.append(e_sb)
            for hh in range(2):
                nc.vector.tensor_tensor(
                    sT_sb[:, hh * 1024:(hh + 1) * 1024],
                    e_sbs[hh],
                    maskT_sb[:, hh * 1024:(hh + 1) * 1024],
                    ALU.mult,
                )
            return sT_sb

        def emit_qk_chunk(wname, bname, dstT, srcT, nb):
            ps = ps2.tile([64, 512], F32, tag="pqk")
            for ei in range(EC):
                nc.tensor.matmul(
                    ps,
                    lhsT=w_sb[wname][:, ei, :],
                    rhs=srcT[:, ei, nb * 512:(nb + 1) * 512],
                    start=(ei == 0),
                    stop=(ei == EC - 1),
                )
            # bias folded into the PSUM->SBUF copy on DVE (per-partition add)
            nc.vector.tensor_scalar(
                dstT[:, nb * 512:(nb + 1) * 512], ps, b_sb[bname], None, ALU.add
            )

        def emit_v_proj(kb):
            psv = ps2.tile([128, H], F32, tag="pv")
            for ei in range(EC):
                nc.tensor.matmul(
                    psv,
                    lhsT=XT[:, ei, kb * 128:(kb + 1) * 128],
                    rhs=w_sb["Wv"][:, ei, :],
                    start=(ei == 0),
                    stop=False,
                )
            nc.tensor.matmul(
                psv,
                lhsT=ones_row[:, 0:128],
                rhs=b_sb["bv"],
                start=False,
                stop=True,
            )
            nc.vector.tensor_copy(vA[:, kb, 0:H], psv)
            nc.vector.memset(vA[:, kb, H:H + 1], 1.0)

        sTs = {}
        with tc.tile_pool(name="ps2", bufs=2, space="PSUM") as ps2:
            for nb in range(4):
                emit_qk_chunk("Wq", "bq", qT, IT, nb)
            for nb in (0, 1):
                emit_qk_chunk("Wk", "bk", kT, XT, nb)
            # scores for the first x-half run while the second half loads
            for ki in range(8):
                sTs[ki] = emit_score(ki)
            for nb in (2, 3):
                emit_qk_chunk("Wk", "bk", kT, XT, nb)
            for kb in range(SC):
                emit_v_proj(kb)

        # ---- ctx accumulation (PSUM banks freed by ps2 close) ----
        psctx = ctx.enter_context(tc.tile_pool(name="psctx", bufs=1, space="PSUM"))

        # [q_within, qj, 64 ctx + 1 denom + pad] — 128-wide regions keep each
        # accumulation group inside one PSUM bank.
        ctxall = psctx.tile([128, SC, 128], F32, tag="ctxall")

        def emit_ctx(ki):
            # start=True zeroes the whole 2KB PSUM bank, so only the first
            # matmul touching each bank (4 qj regions per bank) gets it; the
            # other ki=0 writes land on zeroed-has_written elements and
            # overwrite. stop on the bank's last matmul.
            sT_sb = sTs.pop(ki)
            for qj in range(SC):
                nc.tensor.matmul(
                    ctxall[:, qj, 0:H + 1],
                    lhsT=sT_sb[:, qj * 128:(qj + 1) * 128],
                    rhs=vA[:, ki, 0:H + 1],
                    start=(ki == 0 and qj % 4 == 0),
                    stop=(ki == SC - 1 and qj % 4 == 3),
                )

        for ki in range(8, SC):
            sTs[ki] = emit_score(ki)
            emit_ctx(ki - 8)
        for ki in range(8, SC):
            emit_ctx(ki)

        # vectorized epilogue: one reciprocal over all 16 denominators, one
        # free-dim-broadcast multiply, one 512KB output DMA
        recip_t = outp.tile([128, SC, 1], F32, tag="recip")
        nc.vector.reciprocal(recip_t, ctxall[:, :, H:H + 1])
        recip_bcast = bass.AP(
            tensor=recip_t.tensor,
            offset=recip_t.offset,
            ap=[recip_t.ap[0], recip_t.ap[1], [0, H]],
        )
        o_all = outp.tile([128, SC, H], F32, tag="o")
        nc.vector.tensor_tensor(o_all, ctxall[:, :, 0:H], recip_bcast, ALU.mult)
        nc.sync.dma_start(
            out=dout.ap().rearrange("(qj p) h -> p qj h", p=128), in_=o_all
        )

    nc.compile()
    return nc


def get_program():
    if "nc" not in _cache:
        _cache["nc"] = _build_program()
    return _cache["nc"]


def make_in_maps(I, x, mask, Wq, bq, Wk, bk, Wv, bv):
    I = np.asarray(I, dtype=np.float32)
    x = np.asarray(x, dtype=np.float32)
    mask = np.asarray(mask, dtype=np.int32)
    import ml_dtypes

    BF = ml_dtypes.bfloat16
    Wpack = np.concatenate(
        [
            np.asarray(Wq, dtype=np.float32).astype(BF),
            np.asarray(Wk, dtype=np.float32).astype(BF),
            np.asarray(Wv, dtype=np.float32).astype(BF),
        ],
        axis=1,
    )
    bpack = np.concatenate(
        [
            np.broadcast_to(np.asarray(bq, np.float32).reshape(H, 1), (H, H)),
            np.broadcast_to(np.asarray(bk, np.float32).reshape(H, 1), (H, H)),
        ],
        axis=1,
    ).astype(np.float32)
    bv = np.asarray(bv, dtype=np.float32).reshape(1, H).astype(BF)

    return [
        {
            "IT": np.ascontiguousarray(I[b].T),
            "XT": np.ascontiguousarray(x[b].T),
            "maskT": np.ascontiguousarray(mask[b].T).astype(np.uint8),
            "Wpack": Wpack, "bpack": bpack, "bv": bv,
        }
        for b in range(B)
    ]


def kernel(I, x, mask, Wq, bq, Wk, bk, Wv, bv):
    nc = get_program()
    in_maps = make_in_maps(I, x, mask, Wq, bq, Wk, bk, Wv, bv)
    res = run_bass_kernel_spmd(nc, in_maps, list(range(N_CORES)))
    out = np.stack([res.results[b]["out"] for b in range(B)], axis=0)
    return out.astype(np.float32)



# revision 8
# speedup vs baseline: 1.0481x; 1.0481x over previous
"""Trainium2 Bass kernel for a single DeBERTa-style attention head.

Problem shapes (hardcoded):
  B=8, S=2048, E=768(n_embed), H=64(head)
  q = I @ Wq + bq ; k = x @ Wk + bk ; v = x @ Wv + bv
  w = (q @ k^T) / sqrt(E) ; w = where(mask==0, -1e9, w)
  scores = softmax(w, axis=-1) ; out = scores @ v

Sharding: data-parallel over batch B across the 8 NeuronCores (one batch
element per core, identical SPMD program).

v2 changes vs v1 (v1 measured 104us, bottleneck = serialized fp32 input
DMA with a ~30us dead start before any compute):
  - I^T and x^T are cast to bf16 on the host, halving their HBM bytes
    (12.6MB -> 6.3MB per core) and making them plain copies eligible for
    HWDGE on the sync queue.  The mask uint8->bf16 cast-DMAs stay on the
    gpsimd SWDGE queue, which now streams CONCURRENTLY from t=0 instead
    of queueing behind 12.6MB of fp32 input.
  - Input DMAs are chunked (I^T by 128-row E-chunks, x^T by 512-column
    S-blocks) with the q/k/v projection matmuls ordered to trail the
    stream chunk-by-chunk, so the PE starts ~2us in (also warming the
    PE p-state ramp) instead of waiting for a whole 6.3MB tile.
  - v-projection PSUM->SBUF copies moved to gpsimd to unload DVE.

Per-core dataflow (bf16 operands, fp32 PSUM accumulation):
  1. qT [64,S] = Wq^T I^T accumulated per E-chunk as chunks land; kT
     per 512-col S-block; v per 128-col k-chunk + bias + a ones column
     (v_aug) so the softmax denominator falls out of the ctx matmul's
     extra output column.
  2. k-chunk-major attention: w^T-chunk [128k, q] = kT-chunk^T @ qT;
     e = exp(w^T / sqrt(E)) on ACT straight from PSUM (no row max
     needed: |w/sqrt(E)| is O(1) so exp cannot overflow, and softmax is
     shift-invariant); s^T = e * mask^T (multiplicative masking matches
     the reference's -1e9 additive mask, which underflows to exactly 0
     after softmax); ctx[q-chunk, 0:65] accumulates s^T-chunk^T @ v_aug
     over all 16 k-chunks in PSUM.
  3. out = ctx[:,0:64] * (1/ctx[:,64]).
"""

import math
from contextlib import ExitStack

import numpy as np

import concourse.bass as bass
import concourse.tile as tile
import concourse.mybir as mybir
from concourse import bacc
from concourse.bass_utils import run_bass_kernel_spmd

B, S, E, H = 8, 2048, 768, 64
N_CORES = 8
SC = S // 128   # 16 seq chunks
EC = E // 128   # 6 embed chunks
NB = 4          # 512-col S-blocks
SCALE = 1.0 / math.sqrt(E)

F32 = mybir.dt.float32
BF16 = mybir.dt.bfloat16
U8 = mybir.dt.uint8
AF = mybir.ActivationFunctionType
ALU = mybir.AluOpType

_cache = {}


def _build_program():
    nc = bacc.Bacc("TRN2", target_bir_lowering=False, debug=False)

    dIT = nc.dram_tensor("IT", [E, S], BF16, kind="ExternalInput")
    dXT = nc.dram_tensor("XT", [E, S], BF16, kind="ExternalInput")
    dmT = nc.dram_tensor("maskT", [S, S], U8, kind="ExternalInput")
    dW = nc.dram_tensor("Wpack", [E, 3 * H], BF16, kind="ExternalInput")
    dB = nc.dram_tensor("bpack", [H, 2 * H], F32, kind="ExternalInput")
    dbv = nc.dram_tensor("bv", [1, H], BF16, kind="ExternalInput")
    dout = nc.dram_tensor("out", [S, H], F32, kind="ExternalOutput")

    with tile.TileContext(nc) as tc, ExitStack() as ctx:
        singles = ctx.enter_context(tc.tile_pool(name="singles", bufs=1))

        # --- weights first (tiny, unblock the first matmuls) ---
        w_all = singles.tile([128, EC, 3 * H], BF16, tag="Wpack")
        nc.sync.dma_start(
            out=w_all, in_=dW.ap().rearrange("(ec p) h -> p ec h", p=128)
        )
        b_all = singles.tile([H, 2 * H], F32, tag="bpack")
        nc.sync.dma_start(out=b_all, in_=dB.ap())
        bv_t = singles.tile([1, H], BF16, tag="bv")
        nc.sync.dma_start(out=bv_t, in_=dbv.ap())
        w_sb = {
            "Wq": w_all[:, :, 0:H],
            "Wk": w_all[:, :, H:2 * H],
            "Wv": w_all[:, :, 2 * H:3 * H],
        }
        b_sb = {"bq": b_all[:, 0:1], "bk": b_all[:, H:H + 1], "bv": bv_t}

        ones_row = singles.tile([1, S], BF16, tag="ones")
        nc.vector.memset(ones_row, 1.0)

        # --- bulk input streams ---
        # sync/HWDGE queue: I^T in 6 E-chunks (contiguous 128 rows x 4KB),
        # then x^T in 4 512-col S-blocks (768 rows x 1KB runs).
        IT = singles.tile([128, EC, S], BF16, tag="IT")
        for ei in range(EC):
            nc.sync.dma_start(
                out=IT[:, ei, :], in_=dIT.ap()[ei * 128:(ei + 1) * 128, :]
            )
        XT = singles.tile([128, EC, S], BF16, tag="XT")
        for blk in range(NB):
            nc.sync.dma_start(
                out=XT[:, :, blk * 512:(blk + 1) * 512],
                in_=dXT.ap()[:, blk * 512:(blk + 1) * 512].rearrange(
                    "(ec p) s -> p ec s", p=128
                ),
            )

        # gpsimd/SWDGE queue (concurrent with the above): whole mask^T as
        # uint8 -> bf16 cast-DMAs, 4 quad-chunks to amortize the ~1us
        # fixed SWDGE generation cost per DMA.
        maskT_all = singles.tile([128, SC, S], BF16, tag="maskT")
        for g in range(0, SC, 4):
            nc.gpsimd.dma_start(
                out=maskT_all[:, g:g + 4, :],
                in_=dmT.ap()[g * 128:(g + 4) * 128, :].rearrange(
                    "(t p) q -> p t q", p=128
                ),
            )

        qT = singles.tile([64, S], BF16, tag="qT")
        kT = singles.tile([64, S], BF16, tag="kT")
        vA = singles.tile([128, SC, 66], BF16, tag="vA")
        # ones column for the softmax-denominator trick, set once
        nc.vector.memset(vA[:, :, H:H + 1], 1.0)

        psw = ctx.enter_context(tc.tile_pool(name="psw", bufs=2, space="PSUM"))
        sp = ctx.enter_context(tc.tile_pool(name="sp", bufs=9))
        eep = ctx.enter_context(tc.tile_pool(name="eep", bufs=3))
        outp = ctx.enter_context(tc.tile_pool(name="outp", bufs=1))

        def emit_score(ki):
            """w^T-chunk -> exp -> mask multiply; returns the sT tile.

            Both w halves are emitted before both exps (and both exps
            before both multiplies) so each engine sees its two ops
            back-to-back."""
            maskT_sb = maskT_all[:, ki, :]
            sT_sb = sp.tile([128, S], BF16, tag="sT")
            wps = []
            for hh in range(2):
                wp = psw.tile([128, 1024], F32, tag="w")
                for nb in range(2):
                    nc.tensor.matmul(
                        wp[:, nb * 512:(nb + 1) * 512],
                        lhsT=kT[:, ki * 128:(ki + 1) * 128],
                        rhs=qT[:, (hh * 2 + nb) * 512:(hh * 2 + nb + 1) * 512],
                        start=True,
                        stop=True,
                    )
                wps.append(wp)
            e_sbs = []
            for hh in range(2):
                e_sb = eep.tile([128, 1024], BF16, tag="e")
                nc.scalar.activation(e_sb, wps[hh], AF.Exp, scale=SCALE)
                e_sbs.append(e_sb)
            for hh in range(2):
                nc.vector.tensor_tensor(
                    sT_sb[:, hh * 1024:(hh + 1) * 1024],
                    e_sbs[hh],
                    maskT_sb[:, hh * 1024:(hh + 1) * 1024],
                    ALU.mult,
                )
            return sT_sb

        sTs = {}
        # --- q projection: E-chunk-major, trailing the IT stream.
        # One [64, NB, 512] accumulator spanning 4 banks; each nb slice is
        # one 2KB bank so accumulation groups stay bank-local.
        with tc.tile_pool(name="psQ", bufs=1, space="PSUM") as psQ:
            psqall = psQ.tile([64, NB, 512], F32, tag="pq")
            for ei in range(EC):
                for nb in range(NB):
                    nc.tensor.matmul(
                        psqall[:, nb, :],
                        lhsT=w_sb["Wq"][:, ei, :],
                        rhs=IT[:, ei, nb * 512:(nb + 1) * 512],
                        start=(ei == 0),
                        stop=(ei == EC - 1),
                    )
            for nb in range(NB):
                nc.vector.tensor_scalar(
                    qT[:, nb * 512:(nb + 1) * 512], psqall[:, nb, :],
                    b_sb["bq"], None, ALU.add,
                )

        with tc.tile_pool(name="psA", bufs=2, space="PSUM") as psA:

            def emit_k_block(blk):
                psk = psA.tile([64, 512], F32, tag="pk")
                for ei in range(EC):
                    nc.tensor.matmul(
                        psk,
                        lhsT=w_sb["Wk"][:, ei, :],
                        rhs=XT[:, ei, blk * 512:(blk + 1) * 512],
                        start=(ei == 0),
                        stop=(ei == EC - 1),
                    )
                nc.vector.tensor_scalar(
                    kT[:, blk * 512:(blk + 1) * 512], psk, b_sb["bk"], None,
                    ALU.add,
                )

            def emit_v_proj(kb):
                psv = psA.tile([128, H], F32, tag="pv")
                for ei in range(EC):
                    nc.tensor.matmul(
                        psv,
                        lhsT=XT[:, ei, kb * 128:(kb + 1) * 128],
                        rhs=w_sb["Wv"][:, ei, :],
                        start=(ei == 0),
                        stop=False,
                    )
                nc.tensor.matmul(
                    psv,
                    lhsT=ones_row[:, 0:128],
                    rhs=b_sb["bv"],
                    start=False,
                    stop=True,
                )
                nc.vector.tensor_copy(vA[:, kb, 0:H], psv)

            # k/v for block b+1 emitted before scores for block b so the
            # PE never queues score work (which ACT may backpressure)
            # ahead of projection work whose inputs are already resident.
            emit_k_block(0)
            for kb in range(4):
                emit_v_proj(kb)
            emit_k_block(1)
            for kb in range(4, 8):
                emit_v_proj(kb)
            for ki in range(0, 4):
                sTs[ki] = emit_score(ki)
            emit_k_block(2)
            for kb in range(8, 12):
                emit_v_proj(kb)
            for ki in range(4, 8):
                sTs[ki] = emit_score(ki)
            emit_k_block(3)
            for kb in range(12, 16):
                emit_v_proj(kb)

        # ---- ctx accumulation (psA's 4 banks freed above) ----
        psctx = ctx.enter_context(
            tc.tile_pool(name="psctx", bufs=1, space="PSUM")
        )

        # [q_within, qj, 64 ctx + 1 denom + pad] — 128-wide regions keep
        # each accumulation group inside one PSUM bank.
        ctxall = psctx.tile([128, SC, 128], F32, tag="ctxall")

        def emit_ctx(ki):
            # start=True zeroes the whole 2KB PSUM bank, so only the first
            # matmul touching each bank (4 qj regions per bank) gets it;
            # stop on the bank's last matmul.
            sT_sb = sTs.pop(ki)
            for qj in range(SC):
                nc.tensor.matmul(
                    ctxall[:, qj, 0:H + 1],
                    lhsT=sT_sb[:, qj * 128:(qj + 1) * 128],
                    rhs=vA[:, ki, 0:H + 1],
                    start=(ki == 0 and qj % 4 == 0),
                    stop=(ki == SC - 1 and qj % 4 == 3),
                )

        for ki in range(8, SC):
            sTs[ki] = emit_score(ki)
            emit_ctx(ki - 8)
        for ki in range(8, SC):
            emit_ctx(ki)

        # vectorized epilogue: one reciprocal over all 16 denominators,
        # one free-dim-broadcast multiply, one 512KB output DMA
        recip_t = outp.tile([128, SC, 1], F32, tag="recip")
        nc.vector.reciprocal(recip_t, ctxall[:, :, H:H + 1])
        recip_bcast = bass.AP(
            tensor=recip_t.tensor,
            offset=recip_t.offset,
            ap=[recip_t.ap[0], recip_t.ap[1], [0, H]],
        )
        o_all = outp.tile([128, SC, H], F32, tag="o")
        nc.vector.tensor_tensor(o_all, ctxall[:, :, 0:H], recip_bcast, ALU.mult)
        nc.sync.dma_start(
            out=dout.ap().rearrange("(qj p) h -> p qj h", p=128), in_=o_all
        )

    nc.compile()
    return nc


def get_program():
    if "nc" not in _cache:
        _cache["nc"] = _build_program()
    return _cache["nc"]


def make_in_maps(I, x, mask, Wq, bq, Wk, bk, Wv, bv):
    import ml_dtypes

    BF = ml_dtypes.bfloat16
    I = np.asarray(I, dtype=np.float32)
    x = np.asarray(x, dtype=np.float32)
    mask = np.asarray(mask, dtype=np.int32)

    Wpack = np.concatenate(
        [
            np.asarray(Wq, dtype=np.float32).astype(BF),
            np.asarray(Wk, dtype=np.float32).astype(BF),
            np.asarray(Wv, dtype=np.float32).astype(BF),
        ],
        axis=1,
    )
    bpack = np.concatenate(
        [
            np.broadcast_to(np.asarray(bq, np.float32).reshape(H, 1), (H, H)),
            np.broadcast_to(np.asarray(bk, np.float32).reshape(H, 1), (H, H)),
        ],
        axis=1,
    ).astype(np.float32)
    bv = np.asarray(bv, dtype=np.float32).reshape(1, H).astype(BF)

    return [
        {
            "IT": np.ascontiguousarray(I[b].T).astype(BF),
            "XT": np.ascontiguousarray(x[b].T).astype(BF),
            "maskT": np.ascontiguousarray(mask[b].T).astype(np.uint8),
            "Wpack": Wpack, "bpack": bpack, "bv": bv,
        }
        for b in range(B)
    ]


def kernel(I, x, mask, Wq, bq, Wk, bk, Wv, bv):
    nc = get_program()
    in_maps = make_in_maps(I, x, mask, Wq, bq, Wk, bk, Wv, bv)
    res = run_bass_kernel_spmd(nc, in_maps, list(range(N_CORES)))
    out = np.stack([res.results[b]["out"] for b in range(B)], axis=0)
    return out.astype(np.float32)
